# revision 30
# baseline (speedup 1.0000x reference)
"""GNS layer (edge MLP -> segment_sum -> node MLP + layernorms) on 8 trn2 cores.

Sharding: edges partitioned by *receiver* node range; core k owns nodes
[6250k, 6250(k+1)) and the edges whose receiver lands there, sorted by
receiver. Within a core, edges are grouped into 49 blocks of 128 receiver
nodes; the segment-sum is core-local (PSUM one-hot scatter).

Edge pipeline runs "transposed" ([feat, edge] layouts) so the big MLP matmuls
keep weights stationary and stream 512-edge tiles. Sender features arrive via
one transposed dma_gather per superblock region (bulk SWDGE gather instead of
per-tile indirect DMAs). Because dma_gather indices are int16, edges of each
block are split into L (sender < 32768) and H (sender >= 32768) sub-blocks;
the H gather reads a +32768-row-offset view of x with rebased indices.

Host prep (cheap, numpy): sort edges by receiver, L/H split, pad each
(core, block, half) to a uniform tile count so all 8 cores run one SPMD
program; pre-transpose edge_attr and x; fold the edge-layernorm affine
(eg, ebeta) exactly into the node-MLP weights.
"""

import sys

sys.path.insert(0, "/opt/trn_rl_repo")

import numpy as np
import ml_dtypes

import concourse.bacc as bacc
import concourse.bass as bass
import concourse.tile as tile
from concourse import mybir
from concourse.bass_utils import run_bass_kernel_spmd

BF16 = ml_dtypes.bfloat16

N, E, D, A, H = 50000, 500000, 128, 64, 128
NC = 8
NLOC = N // NC            # 6250 nodes per core
NB = 49                   # blocks of 128 local nodes (49*128 = 6272 >= 6250)
NPAD = NB * 128           # 6272
XPAD = 50176              # multiple of 128 covering N
SPLIT = 32768             # int16 gather-index boundary
SBN = 8                   # node blocks per superblock (gather granularity)
EPS = 1e-5
GRP = 4                   # edge tiles batched per pointwise group


def _host_prep(x, edge_index, edge_attr, eW1, eb1, eW2, eb2,
               nW1, nb1, nW2, nb2, eg, ebeta, ng, nbeta):
    x = np.asarray(x, np.float32)
    s = np.asarray(edge_index[0], np.int64).astype(np.int32)
    r = np.asarray(edge_index[1], np.int64).astype(np.int32)
    ea = np.asarray(edge_attr, np.float32)

    perm = np.argsort(r, kind="stable")
    rs, ss = r[perm], s[perm]
    eas = ea[perm]

    # per-(core, block) edge ranges, split by sender < SPLIT
    lo = np.empty((NC, NB), np.int64)
    hi = np.empty((NC, NB), np.int64)
    for k in range(NC):
        for j in range(NB):
            a = k * NLOC + j * 128
            b = min(a + 128, (k + 1) * NLOC)
            lo[k, j] = np.searchsorted(rs, a)
            hi[k, j] = np.searchsorted(rs, max(a, b))
    eL = [[None] * NB for _ in range(NC)]
    eH = [[None] * NB for _ in range(NC)]
    cntL = np.zeros((NC, NB), np.int64)
    cntH = np.zeros((NC, NB), np.int64)
    for k in range(NC):
        for j in range(NB):
            e = np.arange(lo[k, j], hi[k, j])
            m = ss[e] < SPLIT
            eL[k][j] = e[m]
            eH[k][j] = e[~m]
            cntL[k, j] = len(eL[k][j])
            cntH[k, j] = len(eH[k][j])

    TL = np.maximum(1, np.ceil(cntL.max(axis=0) / 128).astype(np.int64))
    TH = np.maximum(1, np.ceil(cntH.max(axis=0) / 128).astype(np.int64))

    # slot layout: per superblock: [L blocks][H blocks]
    sbs = [list(range(a, min(a + SBN, NB))) for a in range(0, NB, SBN)]
    vtL = np.zeros(NB, np.int64)
    vtH = np.zeros(NB, np.int64)
    sb_meta = []   # (t0L, ntL, t0H, ntH)
    t = 0
    for sb in sbs:
        t0L = t
        for j in sb:
            vtL[j] = t
            t += TL[j]
        t0H = t
        for j in sb:
            vtH[j] = t
            t += TH[j]
        sb_meta.append((int(t0L), int(t0H - t0L), int(t0H), int(t - t0H)))
    TT = int(t)
    S = TT * 128

    x_pad = np.zeros((XPAD, D), np.float32)
    x_pad[:N] = x
    xg = x_pad.astype(BF16)
    xt = np.ascontiguousarray(x_pad.T).astype(BF16)  # [128, XPAD]

    # per-core edge arrays
    idxw = np.zeros((NC, 128, TT * 8), np.int16)
    rloc = np.full((NC, 128, TT), -1.0, np.float32)
    rlocT = np.full((NC, 1, S), -1.0, np.float32)
    eaT = np.zeros((NC, 65, S), np.float32)
    eaT[:, 64, :] = 1.0                            # bias row (eb1)
    degs = np.zeros((NC, 2, NPAD), np.float32)
    degs[:, 0, :] = 1.0
    for k in range(NC):
        for j in range(NB):
            for (ed, vt0, rebase) in ((eL[k][j], vtL[j], 0),
                                      (eH[k][j], vtH[j], SPLIT)):
                n = len(ed)
                if n == 0:
                    continue
                base = int(vt0) * 128
                sl = ss[ed] - rebase
                rl = (rs[ed] - (k * NLOC + j * 128)).astype(np.float32)
                i = base + np.arange(n)
                # wrapped int16 idx layout (16-partition wrap, replicated x8)
                for g in range(8):
                    idxw[k, g * 16 + i % 16, i // 16] = sl.astype(np.int16)
                rloc[k, i % 128, i // 128] = rl
                rlocT[k, 0, i] = rl
                eaT[k, :64, i] = eas[ed]  # adv-index dims lead: (n, 64)
        dg = np.bincount(rs[np.searchsorted(rs, k * NLOC):
                            np.searchsorted(rs, (k + 1) * NLOC)] - k * NLOC,
                         minlength=NPAD)
        degs[k, 1, :] = dg[:NPAD]

    # pre-built one-hots: oh_d[e, t, n] = (rloc[e, t] == n); ohT_d[n, i] = (rlocT[i] == n)
    ar = np.arange(128, dtype=np.float32)
    oh_d = (rloc[:, :, :, None] == ar).astype(BF16)          # [NC, 128, TT, 128]
    ohT_d = (rlocT[:, 0, None, :] == ar[:, None]).astype(BF16)  # [NC, 128, S]

    eW1 = np.asarray(eW1, np.float32)
    nW1 = np.asarray(nW1, np.float32)
    wec = np.concatenate([eW1[2 * D:], np.asarray(eb1, np.float32)[None, :]], 0)
    nW1b_eff = np.asarray(eg, np.float32)[:, None] * nW1[D:]
    c_row = np.asarray(ebeta, np.float32) @ nW1[D:]
    nbc = np.stack([np.asarray(nb1, np.float32), c_row], 0)  # [2, H]

    iotam = np.broadcast_to(np.arange(128, dtype=np.float32), (128, 128)).copy()
    idn = np.eye(128, dtype=np.float32)

    common = dict(
        xg=xg,
        wea=eW1[:D].astype(BF16), web=eW1[D:2 * D].astype(BF16),
        wec=wec.astype(BF16), we2=np.asarray(eW2, np.float32).astype(BF16),
        eb2c=np.asarray(eb2, np.float32)[:, None].copy(),   # [128, 1] f32
        wna=nW1[:D].astype(BF16), wnb=nW1b_eff.astype(BF16),
        nbc=nbc.astype(BF16),
        wn2=np.asarray(nW2, np.float32).astype(BF16),
        nb2r=np.asarray(nb2, np.float32)[None, :].astype(BF16),  # [1, D]
        ngm=np.broadcast_to(np.asarray(ng, np.float32), (128, 128)).copy(),
        nbm=np.broadcast_to(np.asarray(nbeta, np.float32), (128, 128)).copy(),
        iotam=iotam.astype(BF16), idn=idn.astype(BF16),
        iotac=np.arange(128, dtype=np.float32)[:, None].copy(),  # [128, 1]
        ones1=np.ones((1, 128), BF16),
        onesw=np.ones((1, GRP * 128), BF16),
        epsc=np.full((128, 1), EPS, np.float32),
    )
    in_maps = []
    for k in range(NC):
        m = dict(common)
        m["xtl"] = np.ascontiguousarray(xt[:, k * NLOC:k * NLOC + NPAD])
        m["xfl"] = np.ascontiguousarray(x_pad[k * NLOC:k * NLOC + NPAD])
        m["idxw"] = idxw[k]
        m["ohd"] = oh_d[k]
        m["ohtd"] = ohT_d[k]
        m["ea"] = eaT[k].astype(BF16)
        m["degs"] = degs[k].astype(BF16)
        in_maps.append(m)
    meta = dict(TL=[int(v) for v in TL], TH=[int(v) for v in TH],
                vtL=[int(v) for v in vtL], vtH=[int(v) for v in vtH],
                sbs=sbs, sb_meta=sb_meta, TT=TT)
    return in_maps, meta


def build_program(nc, meta):
    dt = mybir.dt
    TL, TH = meta["TL"], meta["TH"]
    vtL, vtH = meta["vtL"], meta["vtH"]
    sbs, sb_meta = meta["sbs"], meta["sb_meta"]
    TT = meta["TT"]
    S = TT * 128
    Tmax = int(max(max(TL), max(TH)))
    maxLnt = max(m[1] for m in sb_meta)
    maxHnt = max(m[3] for m in sb_meta)

    def din(name, shape, dtype):
        return nc.dram_tensor(name, shape, dtype, kind="ExternalInput").ap()

    xg = din("xg", [XPAD, D], dt.bfloat16)
    idxw = din("idxw", [128, TT * 8], dt.int16)
    ohd = din("ohd", [128, TT, 128], dt.bfloat16)
    ohtd = din("ohtd", [128, S], dt.bfloat16)
    ea = din("ea", [65, S], dt.bfloat16)
    wea = din("wea", [128, H], dt.bfloat16)
    web = din("web", [128, H], dt.bfloat16)
    wec = din("wec", [65, H], dt.bfloat16)
    we2 = din("we2", [128, H], dt.bfloat16)
    eb2c = din("eb2c", [128, 1], dt.float32)
    wna = din("wna", [128, H], dt.bfloat16)
    wnb = din("wnb", [128, H], dt.bfloat16)
    nbc = din("nbc", [2, H], dt.bfloat16)
    wn2 = din("wn2", [128, D], dt.bfloat16)
    nb2r = din("nb2r", [1, D], dt.bfloat16)
    degs = din("degs", [2, NPAD], dt.bfloat16)
    ngm = din("ngm", [128, 128], dt.float32)
    nbm = din("nbm", [128, 128], dt.float32)
    iotam = din("iotam", [128, 128], dt.bfloat16)
    idn = din("idn", [128, 128], dt.bfloat16)
    iotac = din("iotac", [128, 1], dt.float32)
    ones1 = din("ones1", [1, 128], dt.bfloat16)
    onesw = din("onesw", [1, GRP * 128], dt.bfloat16)
    epsc = din("epsc", [128, 1], dt.float32)
    xtl = din("xtl", [128, NPAD], dt.bfloat16)
    xfl = din("xfl", [NPAD, D], dt.float32)
    out = nc.dram_tensor("out", [NPAD, D], dt.float32, kind="ExternalOutput").ap()

    with tile.TileContext(nc) as tc:
        with (
            tc.tile_pool(name="singles", bufs=1) as singles,
            tc.tile_pool(name="aggp", bufs=1) as aggp,
            tc.tile_pool(name="gbufL", bufs=2) as gbufL,
            tc.tile_pool(name="gbufH", bufs=2) as gbufH,
            tc.tile_pool(name="blockin", bufs=2) as blockin,
            tc.tile_pool(name="work", bufs=3) as work,
            tc.tile_pool(name="nwork", bufs=3) as nwork,
            tc.tile_pool(name="ph1", bufs=2, space="PSUM") as ph1,
            tc.tile_pool(name="ph2", bufs=2, space="PSUM") as ph2,
            tc.tile_pool(name="ptr", bufs=2, space="PSUM") as ptr,
            tc.tile_pool(name="pagg", bufs=1, space="PSUM") as pagg,
        ):
            def load(ap_, shape, dtype, tag):
                t = singles.tile(shape, dtype, tag=tag)
                nc.sync.dma_start(out=t[:], in_=ap_)
                return t

            s_idxw = load(idxw, [128, TT * 8], dt.int16, "idxw")
            s_wea = load(wea, [128, H], dt.bfloat16, "wea")
            s_web = load(web, [128, H], dt.bfloat16, "web")
            s_wec = load(wec, [65, H], dt.bfloat16, "wec")
            s_we2 = load(we2, [128, H], dt.bfloat16, "we2")
            s_eb2c = load(eb2c, [128, 1], dt.float32, "eb2c")
            s_wna = load(wna, [128, H], dt.bfloat16, "wna")
            s_wnb = load(wnb, [128, H], dt.bfloat16, "wnb")
            s_nbc = load(nbc, [2, H], dt.bfloat16, "nbc")
            s_wn2 = load(wn2, [128, D], dt.bfloat16, "wn2")
            s_nb2r = load(nb2r, [1, D], dt.bfloat16, "nb2r")
            s_degs = load(degs, [2, NPAD], dt.bfloat16, "degs")
            s_ngm = load(ngm, [128, 128], dt.float32, "ngm")
            s_nbm = load(nbm, [128, 128], dt.float32, "nbm")
            s_iota = load(iotam, [128, 128], dt.bfloat16, "iotam")
            s_idn = load(idn, [128, 128], dt.bfloat16, "idn")
            s_iotac = load(iotac, [128, 1], dt.float32, "iotac")
            s_ones1 = load(ones1, [1, 128], dt.bfloat16, "ones1")
            s_onesw = load(onesw, [1, GRP * 128], dt.bfloat16, "onesw")
            s_eps = load(epsc, [128, 1], dt.float32, "epsc")
            s_xtl = load(xtl, [128, NPAD], dt.bfloat16, "xtl")

            s_agg = aggp.tile([128, NPAD], dt.bfloat16)   # agg^T
            s_zb = aggp.tile([128, NPAD], dt.bfloat16, tag="zb")  # (x@web)^T rows

            # ---- zb precompute: zb[j] = x_block @ web, 4 blocks per PSUM tile
            for j0 in range(0, NB, 4):
                nj = min(4, NB - j0)
                p4 = ph2.tile([128, GRP * 128], dt.float32, tag="h2")
                for b in range(nj):
                    nc.tensor.matmul(
                        out=p4[:, b * 128:(b + 1) * 128],
                        lhsT=s_xtl[:, (j0 + b) * 128:(j0 + b + 1) * 128],
                        rhs=s_web[:], start=True, stop=True)
                nc.scalar.copy(out=s_zb[:, j0 * 128:(j0 + nj) * 128],
                               in_=p4[:, :nj * 128])

            # ================= edge phase =================
            xg_h = xg.tensor.ap()[SPLIT:, :]
            for si, sb in enumerate(sbs):
                t0L, ntL, t0H, ntH = sb_meta[si]
                xsl = gbufL.tile([128, maxLnt * 128], dt.bfloat16, tag="xsl")
                hL = (ntL + 1) // 2
                for (a, b) in ((0, hL), (hL, ntL)):
                    nc.gpsimd.dma_gather(
                        xsl[:, None, a * 128:b * 128], xg,
                        s_idxw[:, (t0L + a) * 8:(t0L + b) * 8],
                        (b - a) * 128, (b - a) * 128, D, transpose=True,
                        single_packet=False, queue_num=0)
                xsh = gbufH.tile([128, maxHnt * 128], dt.bfloat16, tag="xsh")
                hH = (ntH + 1) // 2
                for (a, b) in ((0, hH), (hH, ntH)):
                    nc.gpsimd.dma_gather(
                        xsh[:, None, a * 128:b * 128], xg_h,
                        s_idxw[:, (t0H + a) * 8:(t0H + b) * 8],
                        (b - a) * 128, (b - a) * 128, D, transpose=True,
                        single_packet=False, queue_num=0)

                for j in sb:
                    p_agg = pagg.tile([128, 128], dt.float32, tag="agg")
                    ntot = TL[j] + TH[j]
                    tc_i = 0
                    for (vt0, Tv, xbuf, bt0) in ((vtL[j], TL[j], xsl, t0L),
                                                 (vtH[j], TH[j], xsh, t0H)):
                        eab = blockin.tile([65, Tmax * 128], dt.bfloat16, tag="eab")
                        nc.sync.dma_start(out=eab[:, :Tv * 128],
                                          in_=ea[:, vt0 * 128:(vt0 + Tv) * 128])
                        ohb = blockin.tile([128, Tmax, 128], dt.bfloat16,
                                           tag="ohb")
                        nc.sync.dma_start(out=ohb[:, :Tv, :],
                                          in_=ohd[:, vt0:vt0 + Tv, :])
                        ohtb = blockin.tile([128, Tmax * 128], dt.bfloat16,
                                            tag="ohtb")
                        nc.sync.dma_start(out=ohtb[:, :Tv * 128],
                                          in_=ohtd[:, vt0 * 128:(vt0 + Tv) * 128])
                        tloc = vt0 - bt0

                        for q0 in range(0, Tv, GRP):
                            nq = min(GRP, Tv - q0)
                            F = nq * 128

                            tr = ptr.tile([128, 8, 128], dt.bfloat16, tag="tr")

                            # h1^T = wea^T xs^T + zb^T onehot^T + wec^T ea
                            p_h1 = ph1.tile([128, GRP * 128], dt.float32, tag="h1")
                            nc.tensor.matmul(
                                out=p_h1[:, :F], lhsT=s_wea[:],
                                rhs=xbuf[:, tloc * 128 + q0 * 128:
                                         tloc * 128 + q0 * 128 + F],
                                start=True, stop=False)
                            nc.tensor.matmul(
                                out=p_h1[:, :F],
                                lhsT=s_zb[:, j * 128:(j + 1) * 128],
                                rhs=ohtb[:, q0 * 128:q0 * 128 + F],
                                start=False, stop=False)
                            nc.tensor.matmul(
                                out=p_h1[:, :F], lhsT=s_wec[:],
                                rhs=eab[:, q0 * 128:q0 * 128 + F],
                                start=False, stop=True)
                            h1r = work.tile([128, GRP * 128], dt.bfloat16, tag="h1r")
                            nc.scalar.activation(
                                out=h1r[:, :F], in_=p_h1[:, :F],
                                func=mybir.ActivationFunctionType.Relu)

                            # h2^T = we2^T relu(h1^T); relu(+eb2) on copy-out
                            p_h2 = ph2.tile([128, GRP * 128], dt.float32, tag="h2")
                            nc.tensor.matmul(out=p_h2[:, :F], lhsT=s_we2[:],
                                             rhs=h1r[:, :F], start=True, stop=True)
                            rT = work.tile([128, GRP * 128], dt.bfloat16, tag="rT")
                            nc.scalar.activation(
                                out=rT[:, :F], in_=p_h2[:, :F],
                                func=mybir.ActivationFunctionType.Relu,
                                bias=s_eb2c[:, 0:1])

                            # per-tile: transpose, LN over features, scatter
                            p_rs = []
                            for q in range(nq):
                                nc.tensor.transpose(
                                    out=tr[:, 4 + q, :],
                                    in_=rT[:, q * 128:(q + 1) * 128],
                                    identity=s_idn[:])
                                p_rs.append(tr[:, 4 + q, :])
                            st = work.tile([128, GRP, 6], dt.float32, tag="st")
                            mv = work.tile([128, GRP, 2], dt.float32, tag="mv")
                            sd = work.tile([128, GRP], dt.float32, tag="sd")
                            inv = work.tile([128, GRP], dt.float32, tag="inv")
                            msg = work.tile([128, GRP * 128], dt.bfloat16, tag="msg")
                            for q in range(nq):
                                nc.vector.bn_stats(out=st[:, q, :], in_=p_rs[q])
                                nc.vector.bn_aggr(out=mv[:, q, :], in_=st[:, q, :])
                            nc.scalar.activation(
                                out=sd[:, :nq], in_=mv[:, :nq, 1],
                                func=mybir.ActivationFunctionType.Sqrt,
                                bias=s_eps[:, 0:1])
                            nc.vector.reciprocal(out=inv[:, :nq],
                                                 in_=sd[:, :nq])
                            for q in range(nq):
                                p_r = p_rs[q]
                                nc.vector.tensor_scalar(
                                    out=msg[:, q * 128:(q + 1) * 128], in0=p_r,
                                    scalar1=mv[:, q, 0:1], scalar2=inv[:, q:q + 1],
                                    op0=mybir.AluOpType.subtract,
                                    op1=mybir.AluOpType.mult)
                                nc.tensor.matmul(
                                    out=p_agg[:],
                                    lhsT=msg[:, q * 128:(q + 1) * 128],
                                    rhs=ohb[:, q0 + q, :],
                                    start=(tc_i == 0), stop=(tc_i == ntot - 1))
                                tc_i += 1
                    nc.scalar.copy(out=s_agg[:, j * 128:(j + 1) * 128], in_=p_agg[:])

            # ================= node phase =================
            for g in range(0, NB, GRP):
                nj = min(GRP, NB - g)
                F = nj * 128
                c0 = g * 128
                p_hn = ph1.tile([128, GRP * 128], dt.float32, tag="h1")
                nc.tensor.matmul(out=p_hn[:, :F], lhsT=s_wna[:],
                                 rhs=s_xtl[:, c0:c0 + F], start=True, stop=False)
                nc.tensor.matmul(out=p_hn[:, :F], lhsT=s_wnb[:],
                                 rhs=s_agg[:, c0:c0 + F], start=False, stop=False)
                nc.tensor.matmul(out=p_hn[:, :F], lhsT=s_nbc[:],
                                 rhs=s_degs[:, c0:c0 + F], start=False, stop=True)
                hnr = nwork.tile([128, GRP * 128], dt.bfloat16, tag="hnr")
                nc.scalar.activation(out=hnr[:, :F], in_=p_hn[:, :F],
                                     func=mybir.ActivationFunctionType.Relu)
                p_up = ph2.tile([128, GRP * 128], dt.float32, tag="h2")
                nc.tensor.matmul(out=p_up[:, :F], lhsT=s_wn2[:],
                                 rhs=hnr[:, :F], start=True, stop=False)
                nc.tensor.matmul(out=p_up[:, :F], lhsT=s_nb2r[:],
                                 rhs=s_onesw[:, :F], start=False, stop=True)
                upT = nwork.tile([128, GRP * 128], dt.bfloat16, tag="upT")
                nc.scalar.copy(out=upT[:, :F], in_=p_up[:, :F])

                xfb = nwork.tile([128, GRP, 128], dt.float32, tag="xfb")
                nc.sync.dma_start(
                    out=xfb[:, :nj, :],
                    in_=xfl[c0:c0 + F, :].rearrange("(q p) f -> p q f", p=128))

                v = nwork.tile([128, GRP, 128], dt.float32, tag="v")
                st = nwork.tile([128, GRP, 6], dt.float32, tag="nst")
                mv = nwork.tile([128, GRP, 2], dt.float32, tag="nmv")
                sd = nwork.tile([128, GRP], dt.float32, tag="nsd")
                inv = nwork.tile([128, GRP], dt.float32, tag="ninv")
                nrm = nwork.tile([128, GRP, 128], dt.float32, tag="nrm")
                of = nwork.tile([128, GRP, 128], dt.float32, tag="of")
                trn = ptr.tile([128, 8, 128], dt.bfloat16, tag="tr")
                for q in range(nj):
                    nc.tensor.transpose(out=trn[:, q, :],
                                        in_=upT[:, q * 128:(q + 1) * 128],
                                        identity=s_idn[:])
                    nc.vector.tensor_tensor(out=v[:, q, :], in0=trn[:, q, :],
                                            in1=xfb[:, q, :],
                                            op=mybir.AluOpType.add)
                    nc.vector.bn_stats(out=st[:, q, :], in_=v[:, q, :])
                    nc.vector.bn_aggr(out=mv[:, q, :], in_=st[:, q, :])
                    nc.scalar.activation(out=sd[:, q:q + 1], in_=mv[:, q, 1:2],
                                         func=mybir.ActivationFunctionType.Sqrt,
                                         bias=s_eps[:, 0:1])
                    nc.vector.reciprocal(out=inv[:, q:q + 1], in_=sd[:, q:q + 1])
                    nc.vector.tensor_scalar(
                        out=nrm[:, q, :], in0=v[:, q, :],
                        scalar1=mv[:, q, 0:1], scalar2=inv[:, q:q + 1],
                        op0=mybir.AluOpType.subtract, op1=mybir.AluOpType.mult)
                nc.vector.tensor_tensor(
                    out=nrm[:, :nj, :], in0=nrm[:, :nj, :],
                    in1=s_ngm[:, None, :].broadcast_to([128, nj, 128]),
                    op=mybir.AluOpType.mult)
                nc.vector.tensor_tensor(
                    out=of[:, :nj, :], in0=nrm[:, :nj, :],
                    in1=s_nbm[:, None, :].broadcast_to([128, nj, 128]),
                    op=mybir.AluOpType.add)
                nc.sync.dma_start(
                    out=out[c0:c0 + F, :].rearrange("(q p) f -> p q f", p=128),
                    in_=of[:, :nj, :])
    return nc


def kernel(x, edge_index, edge_attr, eW1, eb1, eW2, eb2,
           nW1, nb1, nW2, nb2, eg, ebeta, ng, nbeta, _trace=False, _tmpdir=None):
    in_maps, meta = _host_prep(x, edge_index, edge_attr, eW1, eb1, eW2, eb2,
                               nW1, nb1, nW2, nb2, eg, ebeta, ng, nbeta)
    nc = bacc.Bacc("TRN2", target_bir_lowering=False, debug=False)
    build_program(nc, meta)
    nc.compile()
    res = run_bass_kernel_spmd(nc, in_maps, list(range(NC)), tmpdir=_tmpdir,
                               trace=_trace, trace_cores=[0] if _trace else None)
    outs = [res.results[k]["out"][:NLOC] for k in range(NC)]
    full = np.concatenate(outs, axis=0).astype(np.float32)
    kernel._last_results = res
    return full


# revision 33
# speedup vs baseline: 1.0459x; 1.0459x over previous
"""GNS layer (edge MLP -> segment_sum -> node MLP + layernorms) on 8 trn2 cores.

Sharding: edges partitioned by *receiver* node range; core k owns nodes
[6250k, 6250(k+1)) and the edges whose receiver lands there, sorted by
receiver. Within a core, edges are grouped into 49 blocks of 128 receiver
nodes; the segment-sum is core-local (PSUM one-hot scatter).

Edge pipeline runs "transposed" ([feat, edge] layouts) so the big MLP matmuls
keep weights stationary and stream 512-edge tiles. Sender features arrive via
one transposed dma_gather per superblock region (bulk SWDGE gather instead of
per-tile indirect DMAs). Because dma_gather indices are int16, edges of each
block are split into L (sender < 32768) and H (sender >= 32768) sub-blocks;
the H gather reads a +32768-row-offset view of x with rebased indices.

Host prep (cheap, numpy): sort edges by receiver, L/H split, pad each
(core, block, half) to a uniform tile count so all 8 cores run one SPMD
program; pre-transpose edge_attr and x; fold the edge-layernorm affine
(eg, ebeta) exactly into the node-MLP weights.
"""

import sys

sys.path.insert(0, "/opt/trn_rl_repo")

import numpy as np
import ml_dtypes

import concourse.bacc as bacc
import concourse.bass as bass
import concourse.tile as tile
from concourse import mybir
from concourse.bass_utils import run_bass_kernel_spmd

BF16 = ml_dtypes.bfloat16

N, E, D, A, H = 50000, 500000, 128, 64, 128
NC = 8
NLOC = N // NC            # 6250 nodes per core
NB = 49                   # blocks of 128 local nodes (49*128 = 6272 >= 6250)
NPAD = NB * 128           # 6272
XPAD = 50176              # multiple of 128 covering N
SPLIT = 32768             # int16 gather-index boundary
SBN = 8                   # node blocks per superblock (gather granularity)
EPS = 1e-5
GRP = 4                   # edge tiles batched per pointwise group


def _host_prep(x, edge_index, edge_attr, eW1, eb1, eW2, eb2,
               nW1, nb1, nW2, nb2, eg, ebeta, ng, nbeta):
    x = np.asarray(x, np.float32)
    s = np.asarray(edge_index[0], np.int64).astype(np.int32)
    r = np.asarray(edge_index[1], np.int64).astype(np.int32)
    ea = np.asarray(edge_attr, np.float32)

    perm = np.argsort(r, kind="stable")
    rs, ss = r[perm], s[perm]
    eas = ea[perm]

    # per-(core, block) edge ranges, split by sender < SPLIT
    lo = np.empty((NC, NB), np.int64)
    hi = np.empty((NC, NB), np.int64)
    for k in range(NC):
        for j in range(NB):
            a = k * NLOC + j * 128
            b = min(a + 128, (k + 1) * NLOC)
            lo[k, j] = np.searchsorted(rs, a)
            hi[k, j] = np.searchsorted(rs, max(a, b))
    eL = [[None] * NB for _ in range(NC)]
    eH = [[None] * NB for _ in range(NC)]
    cntL = np.zeros((NC, NB), np.int64)
    cntH = np.zeros((NC, NB), np.int64)
    for k in range(NC):
        for j in range(NB):
            e = np.arange(lo[k, j], hi[k, j])
            m = ss[e] < SPLIT
            eL[k][j] = e[m]
            eH[k][j] = e[~m]
            cntL[k, j] = len(eL[k][j])
            cntH[k, j] = len(eH[k][j])

    TL = np.maximum(1, np.ceil(cntL.max(axis=0) / 128).astype(np.int64))
    TH = np.maximum(1, np.ceil(cntH.max(axis=0) / 128).astype(np.int64))

    # slot layout: per superblock: [L blocks][H blocks]
    sbs = [list(range(a, min(a + SBN, NB))) for a in range(0, NB, SBN)]
    vtL = np.zeros(NB, np.int64)
    vtH = np.zeros(NB, np.int64)
    sb_meta = []   # (t0L, ntL, t0H, ntH)
    t = 0
    for sb in sbs:
        t0L = t
        for j in sb:
            vtL[j] = t
            t += TL[j]
        t0H = t
        for j in sb:
            vtH[j] = t
            t += TH[j]
        sb_meta.append((int(t0L), int(t0H - t0L), int(t0H), int(t - t0H)))
    TT = int(t)
    S = TT * 128

    x_pad = np.zeros((XPAD, D), np.float32)
    x_pad[:N] = x
    xg = x_pad.astype(BF16)
    xt = np.ascontiguousarray(x_pad.T).astype(BF16)  # [128, XPAD]

    # per-core edge arrays
    idxw = np.zeros((NC, 128, TT * 8), np.int16)
    rloc = np.full((NC, 128, TT), -1.0, np.float32)
    rlocT = np.full((NC, 1, S), -1.0, np.float32)
    eaT = np.zeros((NC, 65, S), np.float32)
    eaT[:, 64, :] = 1.0                            # bias row (eb1)
    degs = np.zeros((NC, 2, NPAD), np.float32)
    degs[:, 0, :] = 1.0
    for k in range(NC):
        for j in range(NB):
            for (ed, vt0, rebase) in ((eL[k][j], vtL[j], 0),
                                      (eH[k][j], vtH[j], SPLIT)):
                n = len(ed)
                if n == 0:
                    continue
                base = int(vt0) * 128
                sl = ss[ed] - rebase
                rl = (rs[ed] - (k * NLOC + j * 128)).astype(np.float32)
                i = base + np.arange(n)
                # wrapped int16 idx layout (16-partition wrap, replicated x8)
                for g in range(8):
                    idxw[k, g * 16 + i % 16, i // 16] = sl.astype(np.int16)
                rloc[k, i % 128, i // 128] = rl
                rlocT[k, 0, i] = rl
                eaT[k, :64, i] = eas[ed]  # adv-index dims lead: (n, 64)
        dg = np.bincount(rs[np.searchsorted(rs, k * NLOC):
                            np.searchsorted(rs, (k + 1) * NLOC)] - k * NLOC,
                         minlength=NPAD)
        degs[k, 1, :] = dg[:NPAD]

    # pre-built one-hots: oh_d[e, t, n] = (rloc[e, t] == n); ohT_d[n, i] = (rlocT[i] == n)
    ar = np.arange(128, dtype=np.float32)
    oh_d = (rloc[:, :, :, None] == ar).astype(BF16)          # [NC, 128, TT, 128]
    ohT_d = (rlocT[:, 0, None, :] == ar[:, None]).astype(BF16)  # [NC, 128, S]

    eW1 = np.asarray(eW1, np.float32)
    nW1 = np.asarray(nW1, np.float32)
    wec = np.concatenate([eW1[2 * D:], np.asarray(eb1, np.float32)[None, :]], 0)
    nW1b_eff = np.asarray(eg, np.float32)[:, None] * nW1[D:]
    c_row = np.asarray(ebeta, np.float32) @ nW1[D:]
    nbc = np.stack([np.asarray(nb1, np.float32), c_row], 0)  # [2, H]

    iotam = np.broadcast_to(np.arange(128, dtype=np.float32), (128, 128)).copy()
    idn = np.eye(128, dtype=np.float32)

    common = dict(
        xg=xg,
        wea=eW1[:D].astype(BF16), web=eW1[D:2 * D].astype(BF16),
        wec=wec.astype(BF16), we2=np.asarray(eW2, np.float32).astype(BF16),
        eb2c=np.asarray(eb2, np.float32)[:, None].copy(),   # [128, 1] f32
        wna=nW1[:D].astype(BF16), wnb=nW1b_eff.astype(BF16),
        nbc=nbc.astype(BF16),
        wn2=np.asarray(nW2, np.float32).astype(BF16),
        nb2r=np.asarray(nb2, np.float32)[None, :].astype(BF16),  # [1, D]
        ngm=np.broadcast_to(np.asarray(ng, np.float32), (128, 128)).copy(),
        nbm=np.broadcast_to(np.asarray(nbeta, np.float32), (128, 128)).copy(),
        iotam=iotam.astype(BF16), idn=idn.astype(BF16),
        iotac=np.arange(128, dtype=np.float32)[:, None].copy(),  # [128, 1]
        ones1=np.ones((1, 128), BF16),
        onesw=np.ones((1, GRP * 128), BF16),
        epsc=np.full((128, 1), EPS, np.float32),
    )
    in_maps = []
    for k in range(NC):
        m = dict(common)
        m["xtl"] = np.ascontiguousarray(xt[:, k * NLOC:k * NLOC + NPAD])
        m["xfl"] = np.ascontiguousarray(x_pad[k * NLOC:k * NLOC + NPAD])
        m["idxw"] = idxw[k]
        m["ohd"] = oh_d[k]
        m["ohtd"] = ohT_d[k]
        m["ea"] = eaT[k].astype(BF16)
        m["degs"] = degs[k].astype(BF16)
        in_maps.append(m)
    meta = dict(TL=[int(v) for v in TL], TH=[int(v) for v in TH],
                vtL=[int(v) for v in vtL], vtH=[int(v) for v in vtH],
                sbs=sbs, sb_meta=sb_meta, TT=TT)
    return in_maps, meta


def build_program(nc, meta):
    dt = mybir.dt
    TL, TH = meta["TL"], meta["TH"]
    vtL, vtH = meta["vtL"], meta["vtH"]
    sbs, sb_meta = meta["sbs"], meta["sb_meta"]
    TT = meta["TT"]
    S = TT * 128
    Tmax = int(max(max(TL), max(TH)))
    maxLnt = max(m[1] for m in sb_meta)
    maxHnt = max(m[3] for m in sb_meta)

    def din(name, shape, dtype):
        return nc.dram_tensor(name, shape, dtype, kind="ExternalInput").ap()

    xg = din("xg", [XPAD, D], dt.bfloat16)
    idxw = din("idxw", [128, TT * 8], dt.int16)
    ohd = din("ohd", [128, TT, 128], dt.bfloat16)
    ohtd = din("ohtd", [128, S], dt.bfloat16)
    ea = din("ea", [65, S], dt.bfloat16)
    wea = din("wea", [128, H], dt.bfloat16)
    web = din("web", [128, H], dt.bfloat16)
    wec = din("wec", [65, H], dt.bfloat16)
    we2 = din("we2", [128, H], dt.bfloat16)
    eb2c = din("eb2c", [128, 1], dt.float32)
    wna = din("wna", [128, H], dt.bfloat16)
    wnb = din("wnb", [128, H], dt.bfloat16)
    nbc = din("nbc", [2, H], dt.bfloat16)
    wn2 = din("wn2", [128, D], dt.bfloat16)
    nb2r = din("nb2r", [1, D], dt.bfloat16)
    degs = din("degs", [2, NPAD], dt.bfloat16)
    ngm = din("ngm", [128, 128], dt.float32)
    nbm = din("nbm", [128, 128], dt.float32)
    iotam = din("iotam", [128, 128], dt.bfloat16)
    idn = din("idn", [128, 128], dt.bfloat16)
    iotac = din("iotac", [128, 1], dt.float32)
    ones1 = din("ones1", [1, 128], dt.bfloat16)
    onesw = din("onesw", [1, GRP * 128], dt.bfloat16)
    epsc = din("epsc", [128, 1], dt.float32)
    xtl = din("xtl", [128, NPAD], dt.bfloat16)
    xfl = din("xfl", [NPAD, D], dt.float32)
    out = nc.dram_tensor("out", [NPAD, D], dt.float32, kind="ExternalOutput").ap()

    with tile.TileContext(nc) as tc:
        with (
            tc.tile_pool(name="singles", bufs=1) as singles,
            tc.tile_pool(name="aggp", bufs=1) as aggp,
            tc.tile_pool(name="gbufL", bufs=2) as gbufL,
            tc.tile_pool(name="gbufH", bufs=2) as gbufH,
            tc.tile_pool(name="blockin", bufs=2) as blockin,
            tc.tile_pool(name="work", bufs=3) as work,
            tc.tile_pool(name="nwork", bufs=3) as nwork,
            tc.tile_pool(name="ph1", bufs=2, space="PSUM") as ph1,
            tc.tile_pool(name="ph2", bufs=2, space="PSUM") as ph2,
            tc.tile_pool(name="ptr", bufs=2, space="PSUM") as ptr,
            tc.tile_pool(name="pagg", bufs=1, space="PSUM") as pagg,
        ):
            def load(ap_, shape, dtype, tag):
                t = singles.tile(shape, dtype, tag=tag)
                nc.sync.dma_start(out=t[:], in_=ap_)
                return t

            s_wea = load(wea, [128, H], dt.bfloat16, "wea")
            s_web = load(web, [128, H], dt.bfloat16, "web")
            s_wec = load(wec, [65, H], dt.bfloat16, "wec")
            s_we2 = load(we2, [128, H], dt.bfloat16, "we2")
            s_eb2c = load(eb2c, [128, 1], dt.float32, "eb2c")
            s_wna = load(wna, [128, H], dt.bfloat16, "wna")
            s_wnb = load(wnb, [128, H], dt.bfloat16, "wnb")
            s_nbc = load(nbc, [2, H], dt.bfloat16, "nbc")
            s_wn2 = load(wn2, [128, D], dt.bfloat16, "wn2")
            s_nb2r = load(nb2r, [1, D], dt.bfloat16, "nb2r")
            s_degs = load(degs, [2, NPAD], dt.bfloat16, "degs")
            s_ngm = load(ngm, [128, 128], dt.float32, "ngm")
            s_nbm = load(nbm, [128, 128], dt.float32, "nbm")
            s_iota = load(iotam, [128, 128], dt.bfloat16, "iotam")
            s_idn = load(idn, [128, 128], dt.bfloat16, "idn")
            s_iotac = load(iotac, [128, 1], dt.float32, "iotac")
            s_ones1 = load(ones1, [1, 128], dt.bfloat16, "ones1")
            s_onesw = load(onesw, [1, GRP * 128], dt.bfloat16, "onesw")
            s_eps = load(epsc, [128, 1], dt.float32, "epsc")
            s_xtl = load(xtl, [128, NPAD], dt.bfloat16, "xtl")
            s_idxw = load(idxw, [128, TT * 8], dt.int16, "idxw")

            s_agg = aggp.tile([128, NPAD], dt.bfloat16)   # agg^T
            s_zb = aggp.tile([128, NPAD], dt.bfloat16, tag="zb")  # (x@web)^T rows

            # ---- zb precompute: zb[j] = x_block @ web, 4 blocks per PSUM tile
            for j0 in range(0, NB, 4):
                nj = min(4, NB - j0)
                p4 = ph2.tile([128, GRP * 128], dt.float32, tag="h2")
                for b in range(nj):
                    nc.tensor.matmul(
                        out=p4[:, b * 128:(b + 1) * 128],
                        lhsT=s_xtl[:, (j0 + b) * 128:(j0 + b + 1) * 128],
                        rhs=s_web[:], start=True, stop=True)
                nc.scalar.copy(out=s_zb[:, j0 * 128:(j0 + nj) * 128],
                               in_=p4[:, :nj * 128])

            # ================= edge phase =================
            xg_h = xg.tensor.ap()[SPLIT:, :]
            for si, sb in enumerate(sbs):
                t0L, ntL, t0H, ntH = sb_meta[si]
                xsl = gbufL.tile([128, maxLnt * 128], dt.bfloat16, tag="xsl")
                hL = (ntL + 1) // 2
                for (a, b) in ((0, hL), (hL, ntL)):
                    nc.gpsimd.dma_gather(
                        xsl[:, None, a * 128:b * 128], xg,
                        s_idxw[:, (t0L + a) * 8:(t0L + b) * 8],
                        (b - a) * 128, (b - a) * 128, D, transpose=True,
                        single_packet=False, queue_num=0)
                xsh = gbufH.tile([128, maxHnt * 128], dt.bfloat16, tag="xsh")
                hH = (ntH + 1) // 2
                for (a, b) in ((0, hH), (hH, ntH)):
                    nc.gpsimd.dma_gather(
                        xsh[:, None, a * 128:b * 128], xg_h,
                        s_idxw[:, (t0H + a) * 8:(t0H + b) * 8],
                        (b - a) * 128, (b - a) * 128, D, transpose=True,
                        single_packet=False, queue_num=0)

                for j0 in range(sb[0], sb[-1] + 1, 2):
                  pj = [j for j in (j0, j0 + 1) if j <= sb[-1]]
                  agg_a = pagg.tile([128, 128], dt.float32, tag="agg")
                  aggs = {pj[0]: agg_a}
                  if len(pj) > 1:
                      agg_b = pagg.tile([128, 128], dt.float32, tag="agg2")
                      aggs[pj[1]] = agg_b
                  for (typ, xbuf, bt0) in ((0, xsl, t0L), (1, xsh, t0H)):
                   for j in pj:
                    p_agg = aggs[j]
                    ntot = TL[j] + TH[j]
                    tc_i = 0 if typ == 0 else TL[j]
                    for (vt0, Tv) in ([(vtL[j], TL[j])] if typ == 0
                                      else [(vtH[j], TH[j])]):
                        eab = blockin.tile([65, Tmax * 128], dt.bfloat16, tag="eab")
                        nc.sync.dma_start(out=eab[:, :Tv * 128],
                                          in_=ea[:, vt0 * 128:(vt0 + Tv) * 128])
                        ohb = blockin.tile([128, Tmax, 128], dt.bfloat16,
                                           tag="ohb")
                        nc.sync.dma_start(out=ohb[:, :Tv, :],
                                          in_=ohd[:, vt0:vt0 + Tv, :])
                        ohtb = blockin.tile([128, Tmax * 128], dt.bfloat16,
                                            tag="ohtb")
                        nc.sync.dma_start(out=ohtb[:, :Tv * 128],
                                          in_=ohtd[:, vt0 * 128:(vt0 + Tv) * 128])
                        tloc = vt0 - bt0

                        for q0 in range(0, Tv, GRP):
                            nq = min(GRP, Tv - q0)
                            F = nq * 128

                            tr = ptr.tile([128, 8, 128], dt.bfloat16, tag="tr")

                            # h1^T = wea^T xs^T + zb^T onehot^T + wec^T ea
                            p_h1 = ph1.tile([128, GRP * 128], dt.float32, tag="h1")
                            nc.tensor.matmul(
                                out=p_h1[:, :F], lhsT=s_wea[:],
                                rhs=xbuf[:, tloc * 128 + q0 * 128:
                                         tloc * 128 + q0 * 128 + F],
                                start=True, stop=False)
                            nc.tensor.matmul(
                                out=p_h1[:, :F],
                                lhsT=s_zb[:, j * 128:(j + 1) * 128],
                                rhs=ohtb[:, q0 * 128:q0 * 128 + F],
                                start=False, stop=False)
                            nc.tensor.matmul(
                                out=p_h1[:, :F], lhsT=s_wec[:],
                                rhs=eab[:, q0 * 128:q0 * 128 + F],
                                start=False, stop=True)
                            h1r = work.tile([128, GRP * 128], dt.bfloat16, tag="h1r")
                            nc.scalar.activation(
                                out=h1r[:, :F], in_=p_h1[:, :F],
                                func=mybir.ActivationFunctionType.Relu)

                            # h2^T = we2^T relu(h1^T); relu(+eb2) on copy-out
                            p_h2 = ph2.tile([128, GRP * 128], dt.float32, tag="h2")
                            nc.tensor.matmul(out=p_h2[:, :F], lhsT=s_we2[:],
                                             rhs=h1r[:, :F], start=True, stop=True)
                            rT = work.tile([128, GRP * 128], dt.bfloat16, tag="rT")
                            nc.scalar.activation(
                                out=rT[:, :F], in_=p_h2[:, :F],
                                func=mybir.ActivationFunctionType.Relu,
                                bias=s_eb2c[:, 0:1])

                            # per-tile: transpose, LN over features, scatter
                            p_rs = []
                            for q in range(nq):
                                nc.tensor.transpose(
                                    out=tr[:, 4 + q, :],
                                    in_=rT[:, q * 128:(q + 1) * 128],
                                    identity=s_idn[:])
                                p_rs.append(tr[:, 4 + q, :])
                            st = work.tile([128, GRP, 6], dt.float32, tag="st")
                            mv = work.tile([128, GRP, 2], dt.float32, tag="mv")
                            sd = work.tile([128, GRP], dt.float32, tag="sd")
                            inv = work.tile([128, GRP], dt.float32, tag="inv")
                            msg = work.tile([128, GRP * 128], dt.bfloat16, tag="msg")
                            for q in range(nq):
                                nc.vector.bn_stats(out=st[:, q, :], in_=p_rs[q])
                                nc.vector.bn_aggr(out=mv[:, q, :], in_=st[:, q, :])
                            nc.scalar.activation(
                                out=sd[:, :nq], in_=mv[:, :nq, 1],
                                func=mybir.ActivationFunctionType.Sqrt,
                                bias=s_eps[:, 0:1])
                            nc.vector.reciprocal(out=inv[:, :nq],
                                                 in_=sd[:, :nq])
                            for q in range(nq):
                                p_r = p_rs[q]
                                nc.vector.tensor_scalar(
                                    out=msg[:, q * 128:(q + 1) * 128], in0=p_r,
                                    scalar1=mv[:, q, 0:1], scalar2=inv[:, q:q + 1],
                                    op0=mybir.AluOpType.subtract,
                                    op1=mybir.AluOpType.mult)
                                nc.tensor.matmul(
                                    out=p_agg[:],
                                    lhsT=msg[:, q * 128:(q + 1) * 128],
                                    rhs=ohb[:, q0 + q, :],
                                    start=(tc_i == 0), stop=(tc_i == ntot - 1))
                                tc_i += 1
                  for j in pj:
                    nc.scalar.copy(out=s_agg[:, j * 128:(j + 1) * 128],
                                   in_=aggs[j][:])

            # ================= node phase =================
            for g in range(0, NB, GRP):
                nj = min(GRP, NB - g)
                F = nj * 128
                c0 = g * 128
                p_hn = ph1.tile([128, GRP * 128], dt.float32, tag="h1")
                nc.tensor.matmul(out=p_hn[:, :F], lhsT=s_wna[:],
                                 rhs=s_xtl[:, c0:c0 + F], start=True, stop=False)
                nc.tensor.matmul(out=p_hn[:, :F], lhsT=s_wnb[:],
                                 rhs=s_agg[:, c0:c0 + F], start=False, stop=False)
                nc.tensor.matmul(out=p_hn[:, :F], lhsT=s_nbc[:],
                                 rhs=s_degs[:, c0:c0 + F], start=False, stop=True)
                hnr = nwork.tile([128, GRP * 128], dt.bfloat16, tag="hnr")
                nc.scalar.activation(out=hnr[:, :F], in_=p_hn[:, :F],
                                     func=mybir.ActivationFunctionType.Relu)
                p_up = ph2.tile([128, GRP * 128], dt.float32, tag="h2")
                nc.tensor.matmul(out=p_up[:, :F], lhsT=s_wn2[:],
                                 rhs=hnr[:, :F], start=True, stop=False)
                nc.tensor.matmul(out=p_up[:, :F], lhsT=s_nb2r[:],
                                 rhs=s_onesw[:, :F], start=False, stop=True)
                upT = nwork.tile([128, GRP * 128], dt.bfloat16, tag="upT")
                nc.scalar.copy(out=upT[:, :F], in_=p_up[:, :F])

                xfb = nwork.tile([128, GRP, 128], dt.float32, tag="xfb")
                nc.sync.dma_start(
                    out=xfb[:, :nj, :],
                    in_=xfl[c0:c0 + F, :].rearrange("(q p) f -> p q f", p=128))

                v = nwork.tile([128, GRP, 128], dt.float32, tag="v")
                st = nwork.tile([128, GRP, 6], dt.float32, tag="nst")
                mv = nwork.tile([128, GRP, 2], dt.float32, tag="nmv")
                sd = nwork.tile([128, GRP], dt.float32, tag="nsd")
                inv = nwork.tile([128, GRP], dt.float32, tag="ninv")
                nrm = nwork.tile([128, GRP, 128], dt.float32, tag="nrm")
                of = nwork.tile([128, GRP, 128], dt.float32, tag="of")
                trn = ptr.tile([128, 8, 128], dt.bfloat16, tag="tr")
                for q in range(nj):
                    nc.tensor.transpose(out=trn[:, q, :],
                                        in_=upT[:, q * 128:(q + 1) * 128],
                                        identity=s_idn[:])
                    nc.vector.tensor_tensor(out=v[:, q, :], in0=trn[:, q, :],
                                            in1=xfb[:, q, :],
                                            op=mybir.AluOpType.add)
                    nc.vector.bn_stats(out=st[:, q, :], in_=v[:, q, :])
                    nc.vector.bn_aggr(out=mv[:, q, :], in_=st[:, q, :])
                    nc.scalar.activation(out=sd[:, q:q + 1], in_=mv[:, q, 1:2],
                                         func=mybir.ActivationFunctionType.Sqrt,
                                         bias=s_eps[:, 0:1])
                    nc.vector.reciprocal(out=inv[:, q:q + 1], in_=sd[:, q:q + 1])
                    nc.vector.tensor_scalar(
                        out=nrm[:, q, :], in0=v[:, q, :],
                        scalar1=mv[:, q, 0:1], scalar2=inv[:, q:q + 1],
                        op0=mybir.AluOpType.subtract, op1=mybir.AluOpType.mult)
                nc.vector.tensor_tensor(
                    out=nrm[:, :nj, :], in0=nrm[:, :nj, :],
                    in1=s_ngm[:, None, :].broadcast_to([128, nj, 128]),
                    op=mybir.AluOpType.mult)
                nc.vector.tensor_tensor(
                    out=of[:, :nj, :], in0=nrm[:, :nj, :],
                    in1=s_nbm[:, None, :].broadcast_to([128, nj, 128]),
                    op=mybir.AluOpType.add)
                nc.sync.dma_start(
                    out=out[c0:c0 + F, :].rearrange("(q p) f -> p q f", p=128),
                    in_=of[:, :nj, :])
    return nc


def kernel(x, edge_index, edge_attr, eW1, eb1, eW2, eb2,
           nW1, nb1, nW2, nb2, eg, ebeta, ng, nbeta, _trace=False, _tmpdir=None):
    in_maps, meta = _host_prep(x, edge_index, edge_attr, eW1, eb1, eW2, eb2,
                               nW1, nb1, nW2, nb2, eg, ebeta, ng, nbeta)
    nc = bacc.Bacc("TRN2", target_bir_lowering=False, debug=False)
    build_program(nc, meta)
    nc.compile()
    res = run_bass_kernel_spmd(nc, in_maps, list(range(NC)), tmpdir=_tmpdir,
                               trace=_trace, trace_cores=[0] if _trace else None)
    outs = [res.results[k]["out"][:NLOC] for k in range(NC)]
    full = np.concatenate(outs, axis=0).astype(np.float32)
    kernel._last_results = res
    return full
